# revision 1
# baseline (speedup 1.0000x reference)
"""Trainium2 Bass kernel for nn_LHFA_76278619177511.

Fused transposed-attention block (LHFA):
  q = dwconv3(conv1x1(x, Wq), Wq_dw)   (same for k from y, v from z)
  attn = softmax(l2norm(q) @ l2norm(k)^T * temp)   per-head [32,32]
  out = Wproj @ (attn @ v) + Wfus @ [x;y;z]

Strategy (per core, pure data-parallel over batch B=8 on 8 cores):
  - The depthwise 3x3 is folded into the 1x1 conv: merged weight
    W9[c,(tap,k)] = dw[c,tap]*W1[c,k], contracted over K=576 in 5
    PE K-steps using zero-padded input copies (A = padded image,
    B = A shifted +1 col, D = A shifted +1 row) so every shifted tap
    is a plain rectangular AP read.
  - q,k are produced directly TRANSPOSED ([n,c], data-stationary
    matmuls) so the per-head Gram matrix q@k^T needs no transposes;
    v is produced in natural layout [c,n] for attn@v.
  - Row norms via ones-matmul over squared qT/kT; softmax math on the
    tiny [256,32] per-head blocks with DVE 32x32 block transposes.
  - Everything bf16 in SBUF, fp32 accumulation in PSUM.
"""

import numpy as np
import ml_dtypes

import bass_rust
import concourse.bass as bass
import concourse.mybir as mybir
from concourse import tile as tile_mod
from concourse.tile import TileContext
from concourse.vector_clock import ScopedClock
from concourse.bass_utils import run_bass_kernel_spmd

BF16 = mybir.dt.bfloat16
F32 = mybir.dt.float32

C = 64          # input channels
DIM = 256       # q/k/v channels
HEADS = 8
H = W = 128
N = H * W       # 16384
PW = W + 2      # padded row length 130
HB = 16         # band height (output rows per band)
NB = H // HB    # 8 bands
TW = (HB + 2) * PW  # 2340 cols per pad tile

# 5 K-step scheme: (src_tile, K, ap_offset, v_row_off, v_col_off)
#   src 0 = AB pair tile (A rows 0:64, B = A<<1col rows 64:128)
#   src 1 = AD pair tile (A rows 0:64, D = A<<1row rows 64:128)
STEPS = [
    (0, 128, 0 * PW + 0),
    (0, 128, 1 * PW + 0),
    (0, 128, 2 * PW + 0),
    (1, 128, 0 * PW + 2),
    (1, 64, 2 * PW + 2),
]
# taps (dy,dx) per step/half for weight merging
STEP_TAPS = [
    [(-1, -1), (-1, 0)],
    [(0, -1), (0, 0)],
    [(1, -1), (1, 0)],
    [(-1, 1), (0, 1)],
    [(1, 1)],
]

_PATCHED = False


def _patch_tile_drain():
    """This walrus build rejects >1 sem wait on a CTRL (Drain) instruction;
    split the TileContext tail-drain waits onto individual nops."""
    global _PATCHED
    if _PATCHED:
        return
    _PATCHED = True

    def _drain_and_barrier(self, tick_clock, wait_clock):
        nc = self.nc
        drain_inst = nc.sync.drain()
        wait_clock.add_sem_waits(
            drain_inst.ins, ScopedClock({None: tick_clock.global_clock})
        )
        si = drain_inst.ins.sync_info
        waits = list(si.on_wait or [])
        if len(waits) > 1:
            si.on_wait = waits[:1]
            for w in waits[1:]:
                nop = nc.sync.nop(nofuse=True, hint="split_wait")
                nop.ins.sync_info = bass_rust.SyncInfo(on_wait=[w], on_update=[])
        nc.all_engine_barrier()
        assert self.sems is not None
        popped = nc._tile_sem_poison_stack.pop()
        assert popped is self._sem_poison
        nc.clear_and_free_semaphores(list(self.sems.allocated().values()))
        nc.all_engine_barrier()

    tile_mod.TileContext._drain_and_barrier = _drain_and_barrier
    try:
        from concourse import tile_utils
        tile_utils.max_sbuf_usage = 208 * 1024
    except Exception:
        pass



def _split_excess_waits(nc, max_waits=1):
    """This walrus build caps sem waits per instruction encoding; hoist
    excess waits onto preceding same-engine NoOps (queues are in-order,
    so a wait satisfied on an earlier instruction orders the later one)."""
    import bass_rust as _br

    ctr = [0]
    for f in nc.m.functions:
        for blk in f.blocks:
            out = []
            for inst in blk.instructions:
                si = inst.sync_info
                waits = list(si.on_wait) if (si and si.on_wait) else []
                if len(waits) > max_waits:
                    keep = waits[:max_waits]
                    extra = waits[max_waits:]
                    si.on_wait = keep
                    for w in extra:
                        ctr[0] += 1
                        nop = _br.InstNoOp(name=f"wsplit-{ctr[0]}", ins=[], outs=[])
                        nop.engine = inst.engine
                        nop.sync_info = _br.SyncInfo(on_wait=[w], on_update=[])
                        try:
                            nc.register_instruction(nop, overwrite=True)
                        except Exception:
                            pass
                        out.append(nop)
                out.append(inst)
            blk.instructions[:] = out


def _merge_w(W1, Wdw):
    """-> [128, 5, 256] float32: [p, s, c] = dw[c, tap(s, p//64)] * W1[c, p%64]."""
    out = np.zeros((128, 5, 256), np.float32)
    W1 = W1[:, :, 0, 0]  # [256, 64]
    for s, taps in enumerate(STEP_TAPS):
        for half, (dy, dx) in enumerate(taps):
            out[half * 64 : (half + 1) * 64, s, :] = (
                Wdw[:, 0, 1 + dy, 1 + dx][:, None] * W1
            ).T
    return out


def _bf(a):
    return np.ascontiguousarray(a).astype(ml_dtypes.bfloat16)


def _build_nc(wq, wk, wv, wprojT, wfusT, temp_cols):
    """Build the Bass module. Weight arrays are merged/transposed fp32."""
    _patch_tile_drain()
    nc = bass.Bass()

    xd = nc.declare_dram_parameter("x", [C, N], BF16, isOutput=False)
    yd = nc.declare_dram_parameter("y", [C, N], BF16, isOutput=False)
    zd = nc.declare_dram_parameter("z", [C, N], BF16, isOutput=False)
    od = nc.declare_dram_parameter("out", [DIM, N], F32, isOutput=True)

    wq_d = nc.inline_tensor(_bf(wq.reshape(128, 5 * 256)), name="wq9")
    wk_d = nc.inline_tensor(_bf(wk.reshape(128, 5 * 256)), name="wk9")
    wv_d = nc.inline_tensor(_bf(wv.reshape(128, 5 * 256)), name="wv9")
    wp_d = nc.inline_tensor(_bf(wprojT), name="wprojT")  # [128, 512]
    wf_d = nc.inline_tensor(_bf(wfusT), name="wfusT")    # [128, 512]
    tc0_d = nc.inline_tensor(np.ascontiguousarray(temp_cols[0]), name="tcol0")
    id_d = nc.inline_tensor(np.eye(128, dtype=ml_dtypes.bfloat16), name="ident")
    tc1_d = nc.inline_tensor(np.ascontiguousarray(temp_cols[1]), name="tcol1")

    with TileContext(nc) as tc:
        import contextlib

        with contextlib.ExitStack() as ctx:
            wpool = ctx.enter_context(tc.tile_pool(name="wpool", bufs=1))
            vpool = ctx.enter_context(tc.tile_pool(name="vpool", bufs=1))
            pads = ctx.enter_context(tc.tile_pool(name="pads", bufs=2))
            qkp = ctx.enter_context(tc.tile_pool(name="qkp", bufs=4))
            smallp = ctx.enter_context(tc.tile_pool(name="smallp", bufs=2))
            p2p = ctx.enter_context(tc.tile_pool(name="p2p", bufs=3))

            # --- weights to SBUF ---
            wq_sb = wpool.tile([128, 5 * 256], BF16, tag="wq")
            wk_sb = wpool.tile([128, 5 * 256], BF16, tag="wk")
            wv_sb = wpool.tile([128, 5 * 256], BF16, tag="wv")
            wp_sb = wpool.tile([128, 512], BF16, tag="wp")
            wf_sb = wpool.tile([128, 512], BF16, tag="wf")
            ident_sb = wpool.tile([128, 128], BF16, tag="ident")
            tcol = [wpool.tile([128, 1], F32, tag=f"tc{i}", name=f"tcol{i}") for i in range(2)]
            nc.scalar.dma_start(out=wq_sb, in_=wq_d[:])
            nc.scalar.dma_start(out=wk_sb, in_=wk_d[:])
            nc.scalar.dma_start(out=wv_sb, in_=wv_d[:])

            # --- persistent state ---
            v_slab = [vpool.tile([128, N], BF16, tag=f"v{mb}", name=f"vslab{mb}") for mb in range(2)]
            p1stack = ctx.enter_context(contextlib.ExitStack())
            ps_qk = p1stack.enter_context(tc.tile_pool(name="ps_qk", bufs=3, space="PSUM"))
            ps_v = p1stack.enter_context(tc.tile_pool(name="ps_v", bufs=3, space="PSUM"))
            ps_acc = p1stack.enter_context(tc.tile_pool(name="ps_acc", bufs=1, space="PSUM"))
            acc1 = ps_acc.tile([128, 512], F32, tag="acc1")
            acc2 = ps_acc.tile([128, 256], F32, tag="acc2")
            par_all = acc1[:, 0:256]
            pgq = acc1[:, 256:512]
            pgk = acc2

            ins_d = [xd, yd, zd]

            # ================= pass 1: bands =================
            for b in range(NB):
                lr0 = 1 if b == 0 else 0
                nr = (HB + 2) - (1 if b == 0 else 0) - (1 if b == NB - 1 else 0)
                ir0 = max(0, HB * b - 1)

                srcs = []  # per input: (AB, AD)
                tile_engs = [
                    (nc.sync, nc.gpsimd),
                    (nc.scalar, nc.sync),
                    (nc.gpsimd, nc.scalar),
                ]
                for ti, td in enumerate(ins_d):
                    nm = "xyz"[ti]
                    eAB, eAD = tile_engs[ti]
                    AB = pads.tile([128, TW], BF16, tag=f"{nm}AB")
                    AD = pads.tile([128, TW], BF16, tag=f"{nm}AD")
                    src_img = td[:].rearrange("p (r c) -> p r c", c=W)[
                        :, ir0 : ir0 + nr, :
                    ]
                    for T, de in ((AB, eAB), (AD, eAD)):
                        view = T[0:64, :].rearrange("p (r c) -> p r c", c=PW)
                        nc.vector.memset(view[:, :, 0:1], 0.0)
                        nc.gpsimd.memset(view[:, :, 129:130], 0.0)
                        if b == 0:
                            nc.gpsimd.memset(view[:, 0:1, :], 0.0)
                        if b == NB - 1:
                            nc.gpsimd.memset(view[:, HB + 1 : HB + 2, :], 0.0)
                        de.dma_start(
                            out=view[:, lr0 : lr0 + nr, 1 : 1 + W], in_=src_img
                        )
                    eAB.dma_start(out=AB[64:128, 0 : TW - 1], in_=AB[0:64, 1:TW])
                    eAD.dma_start(
                        out=AD[64:128, 0 : TW - PW], in_=AD[0:64, PW:TW]
                    )
                    srcs.append((AB, AD))

                # qT/kT convs + attnraw + sumsq per output row
                for hl in range(HB):
                    g = HB * b + hl
                    first, last = g == 0, g == H - 1
                    base = hl * PW
                    pqk_t = ps_qk.tile([128, 512], F32, tag="pqk")
                    pk_t = pqk_t[:, 0:256]
                    pq_t = pqk_t[:, 256:512]
                    for which, (w_sb, p_t) in enumerate(
                        ((wq_sb, pq_t), (wk_sb, pk_t))
                    ):
                        AB, AD = srcs[which]
                        for s, (st, K, off) in enumerate(STEPS):
                            src = (AB, AD)[st]
                            nc.tensor.matmul(
                                p_t,
                                lhsT=src[0:K, bass.ds(base + off, 128)],
                                rhs=w_sb[0:K, bass.ds(s * 256, 256)],
                                start=(s == 0),
                                stop=(s == 4),
                            )
                    cat = qkp.tile([128, 512], BF16, tag="cat")
                    if hl % 2 == 0:
                        nc.scalar.copy(cat, pqk_t)
                    else:
                        nc.vector.tensor_copy(cat, pqk_t)
                    for mb in range(2):
                        qsl = bass.ds(256 + mb * 128, 128)
                        ksl = bass.ds(mb * 128, 128)
                        nc.tensor.matmul(
                            par_all[:, bass.ds(mb * 128, 128)],
                            lhsT=cat[:, qsl],
                            rhs=cat[:, ksl],
                            start=first,
                            stop=last,
                            skip_group_check=True,
                        )
                        nc.tensor.matmul(
                            pgq[:, bass.ds(mb * 128, 128)],
                            lhsT=cat[:, qsl],
                            rhs=cat[:, qsl],
                            start=first,
                            stop=last,
                            skip_group_check=True,
                        )
                        nc.tensor.matmul(
                            pgk[:, bass.ds(mb * 128, 128)],
                            lhsT=cat[:, ksl],
                            rhs=cat[:, ksl],
                            start=first,
                            stop=last,
                            skip_group_check=True,
                        )

                # v conv (natural layout), 4 chunks of 4 rows
                zAB, zAD = srcs[2]
                zviews = [
                    T[:, :].rearrange("p (r c) -> p r c", c=PW) for T in (zAB, zAD)
                ]
                for cc in range(HB // 4):
                    hl0 = 4 * cc
                    for mb in range(2):
                        pv_t = ps_v.tile([128, 512], F32, tag="pv")
                        for s, (st, K, off) in enumerate(STEPS):
                            rs, cs = divmod(off, PW)
                            rhs = zviews[st][0:K, hl0 + rs : hl0 + rs + 4, cs : cs + 128]
                            nc.tensor.matmul(
                                pv_t,
                                lhsT=wv_sb[0:K, bass.ds(s * 256 + mb * 128, 128)],
                                rhs=rhs,
                                start=(s == 0),
                                stop=(s == 4),
                            )
                        dst = v_slab[mb][:, bass.ds((HB * b + hl0) * W, 512)]
                        if mb == 0:
                            nc.scalar.copy(dst, pv_t)
                        else:
                            nc.vector.tensor_copy(dst, pv_t)

            nc.sync.dma_start(out=wp_sb, in_=wp_d[:])
            nc.sync.dma_start(out=wf_sb, in_=wf_d[:])
            nc.sync.dma_start(out=tcol[0], in_=tc0_d[:])
            nc.sync.dma_start(out=tcol[1], in_=tc1_d[:])
            nc.sync.dma_start(out=ident_sb, in_=id_d[:])

            # ================= phase 1.5: softmax on [256, 32] =================
            ar_sb = [smallp.tile([128, 128], F32, tag=f"arsb{mb}", name=f"arsb{mb}") for mb in range(2)]
            nc.scalar.copy(ar_sb[0], par_all[:, 0:128])
            nc.scalar.copy(ar_sb[1], par_all[:, 128:256])
            bd = [smallp.tile([128, 128], BF16, tag=f"bd{mb}", name=f"bdiag{mb}") for mb in range(2)]
            for mb in range(2):
                scr = smallp.tile([128, 128], F32, tag="scr")
                rnq_c = smallp.tile([128, 1], F32, tag="rnq")
                rnk_c = smallp.tile([128, 1], F32, tag="rnk")
                for g_ps, dst in ((pgq, rnq_c), (pgk, rnk_c)):
                    ssum = smallp.tile([128, 1], F32, tag="ssum")
                    nc.vector.tensor_mul(scr, g_ps[:, bass.ds(mb * 128, 128)], ident_sb)
                    nc.vector.reduce_sum(out=ssum, in_=scr, axis=mybir.AxisListType.X)
                    nc.scalar.sqrt(ssum, ssum)
                    nc.vector.tensor_scalar_max(ssum, ssum, 1e-12)
                    nc.vector.reciprocal(dst, ssum)
                rnqt = smallp.tile([128, 1], F32, tag="rnqt")
                nc.vector.tensor_mul(rnqt, rnq_c, tcol[mb])

                hd = smallp.tile([128, 32], F32, tag="hd")
                for i in range(4):
                    nc.vector.tensor_copy(
                        hd[32 * i : 32 * (i + 1), :],
                        ar_sb[mb][32 * i : 32 * (i + 1), bass.ds(32 * i, 32)],
                    )
                hds = smallp.tile([128, 32], F32, tag="hds")
                nc.scalar.activation(
                    hds, hd, mybir.ActivationFunctionType.Copy, bias=0.0, scale=rnqt
                )
                hdT = smallp.tile([128, 32], F32, tag="hdT")
                nc.vector.transpose(hdT, hds)
                hdTs = smallp.tile([128, 32], F32, tag="hdTs")
                nc.scalar.activation(
                    hdTs, hdT, mybir.ActivationFunctionType.Copy, bias=0.0, scale=rnk_c
                )
                hd3 = smallp.tile([128, 32], F32, tag="hd3")
                nc.vector.transpose(hd3, hdTs)
                nmx = smallp.tile([128, 1], F32, tag="nmx")
                nc.vector.reduce_max(
                    out=nmx, in_=hd3, axis=mybir.AxisListType.X, negate=True
                )
                ex = smallp.tile([128, 32], F32, tag="ex")
                nc.scalar.activation(
                    ex, hd3, mybir.ActivationFunctionType.Exp, bias=nmx, scale=1.0
                )
                sm = smallp.tile([128, 1], F32, tag="sm")
                nc.vector.reduce_sum(out=sm, in_=ex, axis=mybir.AxisListType.X)
                rsm = smallp.tile([128, 1], F32, tag="rsm")
                nc.vector.reciprocal(rsm, sm)
                Pt = smallp.tile([128, 32], F32, tag="Pt")
                nc.scalar.activation(
                    Pt, ex, mybir.ActivationFunctionType.Copy, bias=0.0, scale=rsm
                )
                PtT = smallp.tile([128, 32], F32, tag="PtT")
                nc.vector.transpose(PtT, Pt)
                nc.gpsimd.memset(bd[mb], 0.0)
                for i in range(4):
                    nc.vector.tensor_copy(
                        bd[mb][32 * i : 32 * (i + 1), bass.ds(32 * i, 32)],
                        PtT[32 * i : 32 * (i + 1), :],
                    )

            # ================= pass 2: attn@v + proj + fusion =================
            p1stack.close()
            ps_av = ctx.enter_context(tc.tile_pool(name="ps_av", bufs=4, space="PSUM"))
            ps_po = ctx.enter_context(tc.tile_pool(name="ps_po", bufs=4, space="PSUM"))
            def p2_load_av(ch):
                n0 = 512 * ch
                xy_t = p2p.tile([128, 512], BF16, tag="xy", name="xy_t")
                z_t = p2p.tile([64, 512], BF16, tag="zt", name="z_t")
                nc.gpsimd.dma_start(out=xy_t[0:64, :], in_=xd[:, bass.ds(n0, 512)])
                nc.gpsimd.dma_start(out=xy_t[64:128, :], in_=yd[:, bass.ds(n0, 512)])
                nc.gpsimd.dma_start(out=z_t, in_=zd[:, bass.ds(n0, 512)])
                ao = []
                for mb in range(2):
                    pav = ps_av.tile([128, 512], F32, tag="pav", name="pav")
                    nc.tensor.matmul(
                        pav,
                        lhsT=bd[mb],
                        rhs=v_slab[mb][:, bass.ds(n0, 512)],
                        start=True,
                        stop=True,
                    )
                    ao_t = p2p.tile([128, 512], BF16, tag=f"ao{mb}", name="ao_t")
                    if mb == 0:
                        nc.scalar.copy(ao_t, pav)
                    else:
                        nc.vector.tensor_copy(ao_t, pav)
                    ao.append(ao_t)
                return xy_t, z_t, ao

            def p2_proj(ch, xy_t, z_t, ao):
                n0 = 512 * ch
                for mb in range(2):
                    po = ps_po.tile([128, 512], F32, tag="po", name="po")
                    nc.tensor.matmul(
                        po,
                        lhsT=wp_sb[:, bass.ds(0 * 256 + mb * 128, 128)],
                        rhs=ao[0],
                        start=True,
                        stop=False,
                    )
                    nc.tensor.matmul(
                        po,
                        lhsT=wp_sb[:, bass.ds(1 * 256 + mb * 128, 128)],
                        rhs=ao[1],
                        start=False,
                        stop=False,
                    )
                    nc.tensor.matmul(
                        po,
                        lhsT=wf_sb[0:128, bass.ds(mb * 128, 128)],
                        rhs=xy_t,
                        start=False,
                        stop=False,
                    )
                    nc.tensor.matmul(
                        po,
                        lhsT=wf_sb[0:64, bass.ds(256 + mb * 128, 128)],
                        rhs=z_t,
                        start=False,
                        stop=True,
                    )
                    o_t = p2p.tile([128, 512], F32, tag=f"ot{mb}", name="o_t")
                    if mb == 0:
                        nc.scalar.copy(o_t, po)
                    else:
                        nc.vector.tensor_copy(o_t, po)
                    (nc.sync if mb == 0 else nc.scalar).dma_start(
                        out=od[bass.ds(mb * 128, 128), bass.ds(n0, 512)], in_=o_t
                    )

            pending = None
            for ch in range(32):
                cur = p2_load_av(ch)
                if pending is not None:
                    p2_proj(ch - 1, *pending)
                pending = cur
            p2_proj(31, *pending)

    _split_excess_waits(nc)
    return nc


def kernel(**inputs):
    x = np.asarray(inputs["x"], np.float32)
    y = np.asarray(inputs["y"], np.float32)
    z = np.asarray(inputs["z"], np.float32)
    B = x.shape[0]
    assert B == 8

    wq = _merge_w(np.asarray(inputs["Wq"], np.float32), np.asarray(inputs["Wq_dw"], np.float32))
    wk = _merge_w(np.asarray(inputs["Wk"], np.float32), np.asarray(inputs["Wk_dw"], np.float32))
    wv = _merge_w(np.asarray(inputs["Wv"], np.float32), np.asarray(inputs["Wv_dw"], np.float32))

    wproj = np.asarray(inputs["Wproj"], np.float32)[:, :, 0, 0]  # [256,256] out,in
    wprojT = np.zeros((128, 512), np.float32)
    for kb in range(2):
        # [p, kb*256 + m] = Wproj[m, kb*128 + p]
        wprojT[:, kb * 256 : (kb + 1) * 256] = wproj[:, kb * 128 : (kb + 1) * 128].T

    wfus = np.asarray(inputs["Wfus"], np.float32)[:, :, 0, 0]  # [256, 192]
    wfusT = np.zeros((128, 512), np.float32)
    wfusT[:, 0:256] = wfus[:, 0:128].T          # x,y rows
    wfusT[0:64, 256:512] = wfus[:, 128:192].T   # z rows

    temp = np.asarray(inputs["temperature"], np.float32).reshape(HEADS)
    tfull = np.repeat(temp, 32).astype(np.float32)
    temp_cols = [tfull[0:128].reshape(128, 1), tfull[128:256].reshape(128, 1)]

    nc = _build_nc(wq, wk, wv, wprojT, wfusT, temp_cols)

    in_maps = []
    for i in range(B):
        in_maps.append(
            {
                "x": _bf(x[i].reshape(C, N)),
                "y": _bf(y[i].reshape(C, N)),
                "z": _bf(z[i].reshape(C, N)),
            }
        )
    res = run_bass_kernel_spmd(nc, in_maps, list(range(8)))
    out = np.stack(
        [np.asarray(res.results[i]["out"], np.float32).reshape(DIM, H, W) for i in range(B)]
    )
    return out

